# revision 46
# baseline (speedup 1.0000x reference)
"""LlamaAttention (B=2, S=2048, D=2048, H=16) on 8 Trainium2 NeuronCores.

Sharding: batch x head-group. Core c handles batch b = c // 4 and head group
g = c % 4 (4 heads of 128 dims each -> a 512-wide slice of q/k/v space).
Each core computes q/k/v projections for its slice, attention for its 4
heads, and a partial out-projection (contracting only its 512 dv dims).
Host sums the 4 partials per batch and adds the output bias.

v4 design (v1 fp32r ~812us, v2 bf16 ~533us, v3 ~485us):
  - all matmul inputs bf16 (PE issues a [128,128]x[128,512] matmul every
    216ns back-to-back; fp32r took 466ns). PSUM accumulation stays fp32.
  - x and wo resident in SBUF. Weight loads use wide descriptors
    ([128,512] / [128,2048] tiles, stationary operands sliced out of
    them) -- the ~600ns fixed cost per DMA_DIRECT2D dominated the v3
    startup; wk split across the sync+scalar queues, x on gpsimd.
  - the PE queue is kept dense through the scalar-paced exp stretches by
    a single global filler deque: v-projection chunks 2-3, q-projection
    chunks 1-3 and every out-projection block are chopped into
    per-matmul closures and popped 2 per score-slot. The PV/r
    accumulation of head h is interleaved into head h+1's score slots
    and carries across block boundaries.
  - exp fused 4 key-tiles wide: scores land in a 4-bank [128,2048] PSUM
    tile, one ACTIVATE per group (amortizes the 352-elem fixed overhead
    and the per-instruction semaphore 4x). Only valid with an all-ones
    attention mask (ACT bias is per-partition); a general-mask variant
    with per-tile bias is built lazily on first use.
  - softmax denominator: 15 bf16 pairwise/tree adds on DVE, one
    ones-stationary matmul folds the final partition reduction AND the
    broadcast (fp32 PSUM), reciprocal_approx_fast (~0.7us vs 3.4us for
    DVE reciprocal; 51-ULP error is far below the bf16 noise floor),
    normalization fused into the PV-psum eviction.
  - output yT in bf16; host sums the 4 partials per batch in fp32.
"""

import os
import numpy as np
import ml_dtypes

import concourse.bass as bass
import concourse.tile as tile
from concourse import bacc, mybir
from concourse import bass_utils

B, S, D = 2, 2048, 2048
NH, HD = 16, 128
N_CORES = 8
HPC = 4                      # heads per core
E = HPC * HD                 # 512: per-core q/k/v width
SCALE = float(HD) ** -0.5
F32 = mybir.dt.float32
BF16 = mybir.dt.bfloat16

P = 128                      # partition tile
ST = S // P                  # 16 s partition-tiles
DTI = 16                     # d partition-tiles
SB = 512                     # matmul moving-dim block / query block
NBLK = S // SB               # 4 s blocks
MASK_MIN = float(np.finfo(np.float32).min)
NPBF16 = ml_dtypes.bfloat16


def _build(has_bias: bool, mask_trivial: bool):
    nc = bacc.Bacc("TRN2", target_bir_lowering=False, debug=False,
                   num_devices=N_CORES)

    xTd = nc.dram_tensor("xT", [D, S], BF16, kind="ExternalInput").ap()
    wqT = nc.dram_tensor("wqT", [D, E], BF16, kind="ExternalInput").ap()
    wkT = nc.dram_tensor("wkT", [D, E], BF16, kind="ExternalInput").ap()
    wvT = nc.dram_tensor("wvT", [D, E], BF16, kind="ExternalInput").ap()
    woT = nc.dram_tensor("woT", [E, D], BF16, kind="ExternalInput").ap()
    maskT = nc.dram_tensor("maskT", [S], F32, kind="ExternalInput").ap()
    ones2 = nc.dram_tensor("ones2", [P, SB], BF16, kind="ExternalInput").ap()
    if has_bias:
        bqd = nc.dram_tensor("bq", [E], BF16, kind="ExternalInput").ap()
        bkd = nc.dram_tensor("bk", [E], BF16, kind="ExternalInput").ap()
        bvd = nc.dram_tensor("bv", [E], BF16, kind="ExternalInput").ap()
    yT = nc.dram_tensor("yT", [D, S], BF16, kind="ExternalOutput").ap()

    scw = 4 * SB if mask_trivial else 2 * SB   # exp-fusion width
    SPG = scw // SB                            # key tiles per score group
    NG = ST // SPG                             # groups per head-block

    with tile.TileContext(nc) as tc:
        with tc.tile_pool(name="persist", bufs=1) as persist, \
             tc.tile_pool(name="xpool", bufs=1) as xpool:
            kT = [persist.tile([P, S], BF16, name=f"kT{i}", tag=f"kT{i}")
                  for i in range(HPC)]
            vv = [persist.tile([P, E], BF16, name=f"v{i}", tag=f"v{i}")
                  for i in range(ST)]
            mask_sb = persist.tile([P, ST], F32, name="mask_sb", tag="mask")
            ones_sb = persist.tile([P, P], BF16, name="ones_sb", tag="ones")
            # (mask/ones DMAs are emitted after the weight loads: they are
            # first needed deep into the attention phase)
            if has_bias:
                ones_row = persist.tile([1, SB], BF16, name="ones_row",
                                        tag="onesr")
                nc.sync.dma_start(ones_row[:, :], ones2[0:1, :])
                ones_rp = persist.tile([1, P], BF16, name="ones_rp",
                                       tag="onesrp")
                nc.sync.dma_start(ones_rp[:, :], ones2[0:1, 0:P])
                bq_sb = persist.tile([1, E], BF16, name="bq_sb", tag="bq")
                bk_sb = persist.tile([1, E], BF16, name="bk_sb", tag="bk")
                bv_sb = persist.tile([1, E], BF16, name="bv_sb", tag="bv")
                nc.sync.dma_start(bq_sb[:, :], bqd.rearrange("(a e) -> a e", a=1))
                nc.sync.dma_start(bk_sb[:, :], bkd.rearrange("(a e) -> a e", a=1))
                nc.sync.dma_start(bv_sb[:, :], bvd.rearrange("(a e) -> a e", a=1))
            # wo resident: 4 wide tiles [128, 2048], sliced per eo later.
            wo_sb = [persist.tile([P, D], BF16, name=f"wo_{dv}",
                                  tag=f"wo_{dv}") for dv in range(HPC)]

            # x resident, chunk 0 first (k-projection needs it immediately);
            # the rest split across the gpsimd and sync queues.
            xx = [xpool.tile([P, S], BF16, name=f"x{dt}", tag=f"x{dt}")
                  for dt in range(DTI)]


            # ---------------- Phase K: k projection ----------------
            # weight loads use wide [128,512] descriptors; stationary
            # operands are sliced out of them. wk split sync/scalar queues.
            # All weight pools are opened up front in fresh SBUF (wqp first,
            # LIFO) so no weight DMA has a write-after-read hazard against a
            # projection still reading the previous pool's space.
            wqp = tc.tile_pool(name="wqp", bufs=1)
            wqpool = wqp.__enter__()
            wkp = tc.tile_pool(name="wkp", bufs=1)
            wkpool = wkp.__enter__()
            wvp = tc.tile_pool(name="wvp", bufs=1)
            wvpool = wvp.__enter__()
            kvps = tc.tile_pool(name="ps_kv", bufs=8, space="PSUM")
            pskv = kvps.__enter__()
            # queue plan: wk streams on sync alone (PE consumes a dt every
            # 864ns, sync delivers one every ~650ns); x chunk 0 split
            # gpsimd/scalar so both wk and x race ahead of the k-projection.
            wk_sb, wv_sb, wq_sb = [], [], []
            xq0 = [nc.gpsimd, nc.scalar]
            for dt in range(DTI):
                xq0[dt % 2].dma_start(xx[dt][:, 0:SB],
                                      xTd[dt * P:(dt + 1) * P, 0:SB])
            for dt in range(DTI):
                w_t = wkpool.tile([P, E], BF16, name=f"wk_{dt}",
                                  tag=f"wk_{dt}")
                nc.sync.dma_start(w_t[:, :], wkT[dt * P:(dt + 1) * P, :])
                wk_sb.append(w_t)
            # x chunks 1-3: evens gpsimd, odds behind wk on sync
            xq = [nc.gpsimd, nc.sync]
            for dt in range(DTI):
                xq[dt % 2].dma_start(xx[dt][:, SB:S],
                                     xTd[dt * P:(dt + 1) * P, SB:S])
            for dt in range(DTI):
                w_t = wvpool.tile([P, E], BF16, name=f"wv_{dt}",
                                  tag=f"wv_{dt}")
                nc.sync.dma_start(w_t[:, :], wvT[dt * P:(dt + 1) * P, :])
                wv_sb.append(w_t)
            for dt in range(DTI):
                w_t = wqpool.tile([P, E], BF16, name=f"wq_{dt}",
                                  tag=f"wq_{dt}")
                nc.sync.dma_start(w_t[:, :], wqT[dt * P:(dt + 1) * P, :])
                wq_sb.append(w_t)
            nc.sync.dma_start(mask_sb[:, :],
                              maskT.rearrange("(t p) -> p t", p=P))
            nc.sync.dma_start(ones_sb[:, :], ones2[:, 0:P])
            for dv in range(HPC):
                nc.sync.dma_start(wo_sb[dv][:, :],
                                  woT[dv * P:(dv + 1) * P, :])
            with nc.named_scope("proj_k"):
                for ch in range(NBLK):
                    c0 = ch * SB
                    pss = [pskv.tile([P, SB], F32, name="ps_k")
                           for _ in range(HPC)]
                    for dt in range(DTI):
                        for et in range(HPC):
                            nc.tensor.matmul(
                                pss[et][:, :],
                                wk_sb[dt][:, et * P:(et + 1) * P],
                                xx[dt][:, c0:c0 + SB],
                                start=(dt == 0),
                                stop=(dt == DTI - 1 and not has_bias))
                    for et in range(HPC):
                        if has_bias:
                            nc.tensor.matmul(
                                pss[et][:, :],
                                bk_sb[0:1, et * P:(et + 1) * P],
                                ones_row[0:1, 0:SB],
                                start=False, stop=True)
                        nc.scalar.copy(kT[et][:, c0:c0 + SB], pss[et][:, :])

            # ---------------- Phase V: v projection ----------------
            with nc.named_scope("proj_v"):
                for ch in range(NBLK):
                    c0 = ch * SB
                    pss = [pskv.tile([P, E], F32, name="ps_k")
                           for _ in range(HPC)]
                    for dt in range(DTI):
                        for sl in range(HPC):
                            nc.tensor.matmul(
                                pss[sl][:, :],
                                xx[dt][:, c0 + sl * P:c0 + (sl + 1) * P],
                                wv_sb[dt][:, :],
                                start=(dt == 0),
                                stop=(dt == DTI - 1 and not has_bias))
                    for sl in range(HPC):
                        st = ch * HPC + sl
                        if has_bias:
                            nc.tensor.matmul(
                                pss[sl][:, :], ones_rp[0:1, :],
                                bv_sb[0:1, :], start=False, stop=True)
                        nc.scalar.copy(vv[st][:, :], pss[sl][:, :])
            kvps.__exit__(None, None, None)
            wvp.__exit__(None, None, None)
            wkp.__exit__(None, None, None)

            # ---------------- Phase QA: q proj + attention + yout ----------
            with nc.named_scope("attn"), \
                 tc.tile_pool(name="qtn", bufs=2 * HPC) as qpool, \
                 tc.tile_pool(name="otn", bufs=2 * HPC) as opool, \
                 tc.tile_pool(name="expp", bufs=9 * (4 * SB // scw)) as expp, \
                 tc.tile_pool(name="prtp", bufs=10) as prtp, \
                 tc.tile_pool(name="smx", bufs=1) as smx, \
                 tc.tile_pool(name="stage", bufs=2) as stagep, \
                 tc.tile_pool(name="ps_q", bufs=1, space="PSUM") as ps_q, \
                 tc.tile_pool(name="ps_sc", bufs=4 * SB // scw,
                              space="PSUM") as ps_sc, \
                 tc.tile_pool(name="ps_att", bufs=1, space="PSUM") as ps_att:

                def qproj_closures(blk, sink):
                    c0 = blk * SB
                    cls = []
                    for et in range(HPC):
                        hold = {}
                        for dt in range(DTI):
                            def cl(et=et, dt=dt, c0=c0, hold=hold, sink=sink):
                                if dt == 0:
                                    hold["ps"] = ps_q.tile([P, SB], F32,
                                                           name="ps_qt")
                                ps = hold["ps"]
                                nc.tensor.matmul(
                                    ps[:, :],
                                    wq_sb[dt][:, et * P:(et + 1) * P],
                                    xx[dt][:, c0:c0 + SB],
                                    start=(dt == 0),
                                    stop=(dt == DTI - 1 and not has_bias))
                                if dt == DTI - 1:
                                    if has_bias:
                                        nc.tensor.matmul(
                                            ps[:, :],
                                            bq_sb[0:1, et * P:(et + 1) * P],
                                            ones_row[0:1, 0:SB],
                                            start=False, stop=True)
                                    qt = qpool.tile([P, SB], BF16, name="qt")
                                    # DVE evict: keeps the scalar queue free
                                    # for the exp cadence
                                    nc.vector.tensor_copy(qt[:, :], ps[:, :])
                                    sink.append(qt)
                            cls.append(cl)
                    return cls

                def yout_closures(blk, ots):
                    q0 = blk * SB
                    cls = []
                    for eo in range(DTI):
                        hold = {}
                        for dv in range(HPC):
                            def cl(eo=eo, dv=dv, q0=q0, hold=hold, ots=ots):
                                if dv == 0:
                                    hold["ps"] = ps_q.tile([P, SB], F32,
                                                           name="yps")
                                yps = hold["ps"]
                                nc.tensor.matmul(
                                    yps[:, :],
                                    wo_sb[dv][:, eo * P:(eo + 1) * P],
                                    ots[dv][:, :],
                                    start=(dv == 0), stop=(dv == HPC - 1))
                                if dv == HPC - 1:
                                    stg = stagep.tile([P, SB], BF16,
                                                      name="stg")
                                    # evicts mostly on DVE (scalar stays on
                                    # exp cadence); stores split across two
                                    # queues so the final drain isn't serial
                                    if eo % 4 == 3:
                                        nc.scalar.copy(stg[:, :], yps[:, :])
                                    else:
                                        nc.vector.tensor_copy(stg[:, :],
                                                              yps[:, :])
                                    (nc.gpsimd if eo % 2 == 0
                                     else nc.sync).dma_start(
                                        yT[eo * P:(eo + 1) * P,
                                           q0:q0 + SB], stg[:, :])
                            cls.append(cl)
                    return cls

                fillers = []
                fidx = [0]

                def fill(n):
                    while n > 0 and fidx[0] < len(fillers):
                        fillers[fidx[0]]()
                        fidx[0] += 1
                        n -= 1

                def start_pv_r(blk, h, exs, qt, ots):
                    ops = ps_att.tile([P, SB], F32, name="ops")
                    rps = ps_att.tile([P, SB], F32, name="rps")
                    return {"blk": blk, "h": h, "exs": exs, "qt": qt,
                            "ops": ops, "rps": rps, "racc": None, "ots": ots}

                def step_pv(pend, sk):
                    h = pend["h"]
                    big, off = pend["exs"][sk * SB // scw], (sk * SB) % scw
                    nc.tensor.matmul(
                        pend["ops"][:, :],
                        vv[sk][:, h * P:(h + 1) * P],
                        big[:, off:off + SB],
                        start=(sk == 0), stop=(sk == ST - 1))

                def emit_rsum(pend):
                    lvl = []
                    for exb in pend["exs"]:
                        for o in range(0, scw, 2 * SB):
                            pt = prtp.tile([P, SB], BF16, name="prt")
                            nc.vector.tensor_add(pt[:, :],
                                                 exb[:, o:o + SB],
                                                 exb[:, o + SB:o + 2 * SB])
                            lvl.append(pt)
                    while len(lvl) > 1:
                        nxt = []
                        for i in range(0, len(lvl), 2):
                            pt = prtp.tile([P, SB], BF16, name="prt")
                            nc.vector.tensor_add(pt[:, :], lvl[i][:, :],
                                                 lvl[i + 1][:, :])
                            nxt.append(pt)
                        lvl = nxt
                    pend["racc"] = lvl[0]

                def finish_pv_r(pend):
                    # ones-matmul emitted at finish time so the PE queue never
                    # waits on the DVE add tree (it ran during the next head's
                    # scores).
                    nc.tensor.matmul(pend["rps"][:, :], ones_sb[:, :],
                                     pend["racc"][:, :], start=True, stop=True)
                    rbc = smx.tile([P, SB], F32, name="rbc")
                    nc.vector.reciprocal_approx_fast(rbc[:, :],
                                                     pend["rps"][:, :])
                    ot = opool.tile([P, SB], BF16, name="ot")
                    nc.vector.tensor_mul(ot[:, :], pend["ops"][:, :],
                                         rbc[:, :])
                    pend["ots"].append(ot)
                    if pend["h"] == HPC - 1:
                        fillers.extend(yout_closures(pend["blk"], pend["ots"]))

                # q chunk 0 emitted plain; q1 seeds the deque
                qsinks = [[] for _ in range(NBLK)]
                for cl in qproj_closures(0, qsinks[0]):
                    cl()
                fillers.extend(qproj_closures(1, qsinks[1]))

                pend = None
                blk_ots = [[] for _ in range(NBLK)]
                for blk in range(NBLK):
                    if blk + 2 < NBLK:
                        fillers.extend(qproj_closures(blk + 2,
                                                      qsinks[blk + 2]))
                    # the q tiles for this block must be emitted by now
                    while len(qsinks[blk]) < HPC:
                        fill(1)
                    qts = qsinks[blk]
                    for h in range(HPC):
                        exs = []
                        for g in range(NG):
                            ps = ps_sc.tile([P, scw], F32, name="ps_sct")
                            for j in range(SPG):
                                slot = g * SPG + j
                                nc.tensor.matmul(
                                    ps[:, j * SB:(j + 1) * SB],
                                    kT[h][:, slot * P:(slot + 1) * P],
                                    qts[h][:, :],
                                    start=True, stop=True)
                                # prev head's PV chain runs 2-per-slot in the
                                # first half so its psum bank + the normalize
                                # chain drain long before the next head needs
                                # them; fillers take the second half. finish
                                # sits at slot 12: late enough that the DVE
                                # add tree (racc) is done, early enough that
                                # the normalize chain drains before the next
                                # head reuses the psum banks.
                                if pend is not None and slot < ST // 2:
                                    step_pv(pend, 2 * slot)
                                    step_pv(pend, 2 * slot + 1)
                                else:
                                    fill(2)
                                if slot >= ST // 2:
                                    fill(1)
                                if slot == 12 and pend is not None:
                                    finish_pv_r(pend)
                                    pend = None
                            ext = expp.tile([P, scw], BF16, name="ext")
                            if mask_trivial:
                                nc.scalar.activation(
                                    ext[:, :], ps[:, :],
                                    mybir.ActivationFunctionType.Exp,
                                    bias=0.0, scale=1.0)
                            else:
                                for j in range(SPG):
                                    sk = g * SPG + j
                                    nc.scalar.activation(
                                        ext[:, j * SB:(j + 1) * SB],
                                        ps[:, j * SB:(j + 1) * SB],
                                        mybir.ActivationFunctionType.Exp,
                                        bias=mask_sb[:, sk:sk + 1], scale=1.0)
                            exs.append(ext)
                        pend = start_pv_r(blk, h, exs, qts[h], blk_ots[blk])
                        emit_rsum(pend)
                # drain the last head (blk3/h3): no more scores to interleave
                for sk in range(ST):
                    step_pv(pend, sk)
                    fill(2)
                fill(len(fillers))
                finish_pv_r(pend)
                # the final finish just queued yout(blk3): emit it plain
                fill(len(fillers))
            wqp.__exit__(None, None, None)

    nc.compile()
    return nc


_NC_CACHE = {}


def _get_nc(has_bias: bool, mask_trivial: bool):
    key = (has_bias, mask_trivial)
    if key not in _NC_CACHE:
        _NC_CACHE[key] = _build(has_bias, mask_trivial)
    return _NC_CACHE[key]


def kernel(hidden_states, attention_mask, Wq, bq, Wk, bk, Wv, bv, Wo, bo):
    hidden_states = np.asarray(hidden_states, dtype=np.float32)
    attention_mask = np.asarray(attention_mask, dtype=np.float32)
    Wq = np.asarray(Wq, dtype=np.float32)
    Wk = np.asarray(Wk, dtype=np.float32)
    Wv = np.asarray(Wv, dtype=np.float32)
    Wo = np.asarray(Wo, dtype=np.float32)
    bq = np.asarray(bq, dtype=np.float32)
    bk = np.asarray(bk, dtype=np.float32)
    bv = np.asarray(bv, dtype=np.float32)
    bo = np.asarray(bo, dtype=np.float32)

    has_bias = bool(np.any(bq) or np.any(bk) or np.any(bv))
    mask_trivial = bool(np.all(attention_mask == 1.0))
    nc = _get_nc(has_bias, mask_trivial)

    # Host-side sharding prep (cheap numpy work, not on the HW critical path)
    xT = [np.ascontiguousarray(hidden_states[b].T).astype(NPBF16)
          for b in range(B)]
    addmask = [np.ascontiguousarray((1.0 - attention_mask[b]) * MASK_MIN)
               for b in range(B)]
    ones2 = np.ones((P, SB), dtype=NPBF16)
    Wq_s = Wq * SCALE          # fold the 1/sqrt(hd) scale into Wq (and bq)
    in_maps = []
    for c in range(N_CORES):
        b, g = c // 4, c % 4
        sl = slice(g * E, (g + 1) * E)
        im = {
            "xT": xT[b],
            "wqT": np.ascontiguousarray(Wq_s[sl, :].T).astype(NPBF16),
            "wkT": np.ascontiguousarray(Wk[sl, :].T).astype(NPBF16),
            "wvT": np.ascontiguousarray(Wv[sl, :].T).astype(NPBF16),
            "woT": np.ascontiguousarray(Wo[:, sl].T).astype(NPBF16),
            "maskT": addmask[b],
            "ones2": ones2,
        }
        if has_bias:
            im["bq"] = np.ascontiguousarray(bq[sl] * SCALE).astype(NPBF16)
            im["bk"] = np.ascontiguousarray(bk[sl]).astype(NPBF16)
            im["bv"] = np.ascontiguousarray(bv[sl]).astype(NPBF16)
        in_maps.append(im)

    res = bass_utils.run_bass_kernel_spmd(
        nc, in_maps, core_ids=list(range(N_CORES)),
        trace=bool(int(os.environ.get("BASS_KERNEL_TRACE", "0"))))
    kernel.last_results = res

    out = np.empty((B, S, D), dtype=np.float32)
    for b in range(B):
        acc = res.results[b * 4]["yT"].astype(np.float32)
        for g in range(1, 4):
            acc += res.results[b * 4 + g]["yT"].astype(np.float32)
        out[b] = acc.T + bo
    return out
